# revision 4
# baseline (speedup 1.0000x reference)
"""GAT message-passing kernel for Trainium2 (8 NeuronCores, SPMD).

Strategy (dst-partitioned, no collectives):
  - Sort edges by dst on host; core c owns dst nodes [c*6250, (c+1)*6250).
  - Phase A (replicated): feat_aug = h @ [fc_w | A_l | A_r] written to a DRAM
    table G (bf16, rows [feat 256 | el 4]); per-core er table for own nodes.
  - Phase B: per 128-node window, batch-gather edge rows via InstDMAGatherAnt
    (int16 idx; lo/hi table split for the 32768 index limit), compute
    w = max(exp(e), exp(0.2 e)) (== exp(leaky_relu(e))), build a one-hot
    selection matrix per chunk and accumulate messages + softmax denominators
    into PSUM with TensorE matmuls (single-pass softmax: alpha = exp(e)/sum).
  - Phase C: rst = msg/z; out linear (PE transpose + matmul) + bias fold
    (gat_bias @ out_w + out_b precomputed on host) + layernorm; DMA out.
"""
import os
import numpy as np

import concourse.bass as bass
import concourse.bacc as bacc
import concourse.mybir as mybir
import concourse.tile as tile
import concourse.bass_utils as bu
from concourse.bass_utils import run_bass_kernel_spmd
from concourse.masks import make_identity
from concourse.tile_rust import add_dep_helper
from concourse import ap_utils
from concourse._compat import exact_div

# ---------------- constants ----------------
N, E, F, H = 50000, 800000, 64, 4
HF = H * F
NCORES = 8
NPC = N // NCORES            # 6250
P = 128
NWIN = (NPC + P - 1) // P    # 49
LO = 32768                   # int16 index split point
GROWS = 50176                # 49*1024, padded node count for phase A supertiles
GSTRIDE = 384                # bf16 elems per G row (768B, mult of 256B)
GROW_USED = 260              # feat 256 | el 4
ERROWS = 6272                # 49*128
ERSTRIDE = 128               # bf16 elems per er row (256B)
LN_EPS = 1e-5
NEG = 0.2
F32 = mybir.dt.float32
BF16 = mybir.dt.bfloat16
I16 = mybir.dt.int16

# ---------------- walrus DGE patch (vector-indirect DMA support) ------------
_DGE_FLAG = "--dge-levels=vector_dynamic_offsets,dst_reduce"
_orig_bvo = bu.bir_verify_and_optimise

def _patched_bvo(tmpdir, inp="bir.json", outp="file.neff", arch=None, *, dve_root=None):
    orig_run = bu.run_command
    def run2(cmd, **kw):
        cmd = list(cmd)
        cmd.insert(1, _DGE_FLAG)
        return orig_run(cmd, **kw)
    bu.run_command = run2
    try:
        return _orig_bvo(tmpdir, inp, outp, arch, dve_root=dve_root)
    finally:
        bu.run_command = orig_run

bu.bir_verify_and_optimise = _patched_bvo


def dma_gather_relaxed(eng, out_ap, in_ap, idxs_ap, num_idxs, elem_size, elem_step,
                       queue_num=0, single_packet=False):
    """nc.gpsimd.dma_gather minus the elem_size%256 assert (stride must still
    be a multiple of 256B; read length per row may be arbitrary)."""
    assert idxs_ap.dtype == I16
    assert in_ap.space == bass.MemorySpace.DRAM
    assert idxs_ap.space == bass.MemorySpace.SBUF
    assert out_ap.space == bass.MemorySpace.SBUF
    assert ap_utils.ap_is_contiguous(in_ap.ap[1:])
    assert ap_utils.ap_is_contiguous(out_ap.ap[1:])
    assert ap_utils.ap_is_contiguous(idxs_ap.ap[1:])
    assert in_ap.ap[-1][1] == out_ap.ap[-1][1] == elem_size
    assert out_ap.ap[0][1] * out_ap.ap[1][1] == num_idxs, (out_ap.ap, num_idxs)
    assert in_ap.ap[0][0] == elem_step
    stride_bytes = elem_step * mybir.dt.size(in_ap.dtype)
    stride_bytes_256 = exact_div(stride_bytes, 256)
    assert stride_bytes_256 < 256
    _in_ap = eng.lower_ap_dma(in_ap, for_custom_bir_dma=True)
    _idxs_ap = eng.lower_ap(idxs_ap)
    _out_ap = eng.lower_ap(out_ap)
    return eng.add_instruction(
        mybir.InstDMAGatherAnt(
            name=eng.bass.get_next_instruction_name(),
            ins=[*_in_ap, _idxs_ap, eng.lower_val_access(eng.to_reg(num_idxs))],
            outs=[_out_ap],
            transpose=False,
            num_idxs=num_idxs,
            elem_size=elem_size,
            stride_bytes_256=stride_bytes_256,
            gen_mode=0,
            single_packet=single_packet,
            queue_num=queue_num,
            sbuf_tokens_per_rank=0,
            sbuf_free_dim_per_rank=0,
            sbuf_free_dim_pad_per_rank=0,
            sbuf_byte_offset=0,
        )
    )


def _pieces(j0, j1, maxc=8):
    """split chunk range [j0, j1) into pieces of <= maxc chunks (<=1024 idxs)"""
    out = []
    while j0 < j1:
        out.append((j0, min(j0 + maxc, j1)))
        j0 = min(j0 + maxc, j1)
    return out


def build_program(CL, CH):
    CPW = CL + CH
    nc = bacc.Bacc("TRN2", target_bir_lowering=False, debug=False, num_devices=NCORES)

    hT_p = nc.declare_dram_parameter("hT", [F, GROWS], F32, isOutput=False)
    hTo_p = nc.declare_dram_parameter("hTo", [F, ERROWS], F32, isOutput=False)
    Waug_p = nc.declare_dram_parameter("Waug", [F, 264], F32, isOutput=False)
    outw_p = nc.declare_dram_parameter("outw", [HF, F], F32, isOutput=False)
    vecs_p = nc.declare_dram_parameter("vecs", [P, 3, F], F32, isOutput=False)
    srcq_p = nc.declare_dram_parameter("srcq", [P, NWIN * CPW * 8], I16, isOutput=False)
    dstl_p = nc.declare_dram_parameter("dstl", [P, NWIN * CPW * 8], I16, isOutput=False)
    dstf_p = nc.declare_dram_parameter("dstf", [P, NWIN * CPW], F32, isOutput=False)
    out_p = nc.declare_dram_parameter("out", [NWIN * P, F], F32, isOutput=True)

    G = nc.dram_tensor("G", [GROWS, GSTRIDE], BF16)
    ERL = nc.dram_tensor("ERL", [ERROWS, ERSTRIDE], BF16)

    lo_pieces = _pieces(0, CL)
    hi_pieces = _pieces(CL, CPW)
    er_pieces = _pieces(0, CPW)

    with tile.TileContext(nc) as tc:
        with tc.tile_pool(name="const", bufs=1) as cp:
            iota_f = cp.tile([P, CPW, P], F32)
            nc.gpsimd.iota(iota_f[:], pattern=[[0, CPW], [1, P]], base=0,
                           channel_multiplier=0, allow_small_or_imprecise_dtypes=True)
            ident = cp.tile([P, P], F32)
            make_identity(nc, ident[:])
            Waug_t = cp.tile([F, 264], F32)
            nc.sync.dma_start(out=Waug_t[:], in_=Waug_p[:])
            outw_t = cp.tile([P, 2, F], F32)
            nc.sync.dma_start(out=outw_t[:], in_=outw_p[:].rearrange("(k p) f -> p k f", p=P))
            vecs_t = cp.tile([P, 3, F], F32)
            nc.sync.dma_start(out=vecs_t[:], in_=vecs_p[:])
            srcq_t = cp.tile([P, NWIN * CPW * 8], I16)
            nc.sync.dma_start(out=srcq_t[:], in_=srcq_p[:])
            dstl_t = cp.tile([P, NWIN * CPW * 8], I16)
            nc.sync.dma_start(out=dstl_t[:], in_=dstl_p[:])
            dstf_t = cp.tile([P, NWIN * CPW], F32)
            nc.sync.dma_start(out=dstf_t[:], in_=dstf_p[:])

            table_writes = []
            # ---------------- phase A: G table ----------------
            with tc.tile_pool(name="pha", bufs=2) as pa, \
                 tc.tile_pool(name="phaps", bufs=2, space="PSUM") as pap:
                for st in range(GROWS // 1024):
                    ht = pa.tile([F, 1024], F32, tag="ht")
                    nc.sync.dma_start(out=ht[:], in_=hT_p[:, st * 1024:(st + 1) * 1024])
                    stg = pa.tile([P, 8, GROW_USED], BF16, tag="stg")
                    for t in range(8):
                        ps = pap.tile([P, 264], F32, tag="psA")
                        nc.tensor.matmul(out=ps[:], lhsT=ht[:, t * P:(t + 1) * P],
                                         rhs=Waug_t[:], start=True, stop=True)
                        nc.vector.tensor_copy(out=stg[:, t, :], in_=ps[:, 0:GROW_USED])
                    wi = nc.sync.dma_start(
                        out=G[st * 1024:(st + 1) * 1024, 0:GROW_USED].rearrange(
                            "(s p) c -> p s c", p=P),
                        in_=stg[:])
                    table_writes.append(wi)
                # ---------------- phase A2: er table (own nodes) ----------------
                for sg in range(7):
                    hb = pa.tile([F, 896], F32, tag="hb")
                    nc.sync.dma_start(out=hb[:], in_=hTo_p[:, sg * 896:(sg + 1) * 896])
                    stg2 = pa.tile([P, 7, 4], BF16, tag="stg2")
                    for t in range(7):
                        ps2 = pap.tile([P, 4], F32, tag="psA2")
                        nc.tensor.matmul(out=ps2[:], lhsT=hb[:, t * P:(t + 1) * P],
                                         rhs=Waug_t[:, 260:264], start=True, stop=True)
                        nc.vector.tensor_copy(out=stg2[:, t, :], in_=ps2[:])
                    wi = nc.sync.dma_start(
                        out=ERL[sg * 896:(sg + 1) * 896, 0:4].rearrange(
                            "(s p) c -> p s c", p=P),
                        in_=stg2[:])
                    table_writes.append(wi)

            joiner = nc.sync.nop(nofuse=True)
            for wi in table_writes:
                add_dep_helper(joiner.ins, wi.ins, reason="joiner waits on table writes")

            # ---------------- phase B + C ----------------
            with tc.tile_pool(name="phb", bufs=3) as pb, \
                 tc.tile_pool(name="phc", bufs=2) as pc, \
                 tc.tile_pool(name="phbps", bufs=2, space="PSUM") as pwp, \
                 tc.tile_pool(name="phcps", bufs=2, space="PSUM") as pcp:
                for w in range(NWIN):
                    b8 = w * CPW * 8
                    X = pb.tile([P, CPW, GROW_USED], BF16, tag="X")
                    ER = pb.tile([P, CPW, 4], BF16, tag="ER")
                    for pieces, tbl in ((lo_pieces, G[0:LO, 0:GROW_USED]),
                                        (hi_pieces, G[LO:GROWS, 0:GROW_USED])):
                        for (j0, j1) in pieces:
                            g = dma_gather_relaxed(
                                nc.gpsimd, out_ap=X[:, j0:j1, :], in_ap=tbl,
                                idxs_ap=srcq_t[:, b8 + j0 * 8: b8 + j1 * 8],
                                num_idxs=(j1 - j0) * P, elem_size=GROW_USED, elem_step=GSTRIDE)
                            add_dep_helper(g.ins, joiner.ins, reason="gather waits on joiner")
                    for (j0, j1) in er_pieces:
                        g = dma_gather_relaxed(
                            nc.gpsimd, out_ap=ER[:, j0:j1, :], in_ap=ERL[:, 0:4],
                            idxs_ap=dstl_t[:, b8 + j0 * 8: b8 + j1 * 8],
                            num_idxs=(j1 - j0) * P, elem_size=4, elem_step=ERSTRIDE)
                        add_dep_helper(g.ins, joiner.ins, reason="gather waits on joiner")
                    ew = pb.tile([P, CPW, 4], F32, tag="ew")
                    nc.vector.tensor_tensor(out=ew[:], in0=X[:, :, 256:260], in1=ER[:],
                                            op=mybir.AluOpType.add)
                    w1 = pb.tile([P, CPW, 4], F32, tag="w1")
                    nc.scalar.activation(out=w1[:], in_=ew[:],
                                         func=mybir.ActivationFunctionType.Exp)
                    w2 = pb.tile([P, CPW, 4], F32, tag="w2")
                    nc.scalar.activation(out=w2[:], in_=ew[:],
                                         func=mybir.ActivationFunctionType.Exp, scale=NEG)
                    wb = pb.tile([P, CPW, 4], BF16, tag="wb")
                    nc.vector.tensor_tensor(out=wb[:], in0=w1[:], in1=w2[:],
                                            op=mybir.AluOpType.max)
                    sel = pb.tile([P, CPW, P], BF16, tag="sel")
                    nc.vector.tensor_tensor(
                        out=sel[:], in0=iota_f[:],
                        in1=dstf_t[:, w * CPW:(w + 1) * CPW, None].to_broadcast([P, CPW, P]),
                        op=mybir.AluOpType.is_equal)
                    M = pb.tile([P, CPW, GROW_USED], BF16, tag="M")
                    nc.vector.tensor_tensor(
                        out=M[:, :, 0:256].rearrange("p j (h f) -> p j h f", h=H),
                        in0=X[:, :, 0:256].rearrange("p j (h f) -> p j h f", h=H),
                        in1=wb[:, :, :, None].to_broadcast([P, CPW, H, F]),
                        op=mybir.AluOpType.mult)
                    nc.vector.tensor_copy(out=M[:, :, 256:260], in_=wb[:])
                    psw = pwp.tile([P, GROW_USED], F32, tag="psw")
                    for j in range(CPW):
                        nc.tensor.matmul(out=psw[:], lhsT=sel[:, j, :], rhs=M[:, j, :],
                                         start=(j == 0), stop=(j == CPW - 1))
                    # ---- phase C ----
                    zs = pc.tile([P, 4], F32, tag="zs")
                    nc.vector.tensor_scalar(out=zs[:], in0=psw[:, 256:260],
                                            scalar1=1e-30, scalar2=None,
                                            op0=mybir.AluOpType.max)
                    zr = pc.tile([P, 4], F32, tag="zr")
                    nc.vector.reciprocal(out=zr[:], in_=zs[:])
                    zrep = pc.tile([P, H, F], F32, tag="zrep")
                    nc.vector.tensor_copy(out=zrep[:],
                                          in_=zr[:, :, None].to_broadcast([P, H, F]))
                    rstn = pc.tile([P, HF], F32, tag="rstn")
                    nc.vector.tensor_tensor(out=rstn[:].rearrange("p (h f) -> p h f", h=H),
                                            in0=psw[:, 0:256].rearrange("p (h f) -> p h f", h=H),
                                            in1=zrep[:], op=mybir.AluOpType.mult)
                    psx = pcp.tile([P, F], F32, tag="psx")
                    for half in range(2):
                        pst = pcp.tile([P, P], F32, tag="pst")
                        nc.tensor.transpose(out=pst[:], in_=rstn[:, half * P:(half + 1) * P],
                                            identity=ident[:])
                        rT = pc.tile([P, P], F32, tag="rT")
                        nc.vector.tensor_copy(out=rT[:], in_=pst[:])
                        nc.tensor.matmul(out=psx[:], lhsT=rT[:], rhs=outw_t[:, half, :],
                                         start=(half == 0), stop=(half == 1))
                    xt = pc.tile([P, F], F32, tag="xt")
                    nc.vector.tensor_tensor(out=xt[:], in0=psx[:], in1=vecs_t[:, 0, :],
                                            op=mybir.AluOpType.add)
                    s1 = pc.tile([P, 1], F32, tag="s1")
                    nc.vector.tensor_reduce(out=s1[:], in_=xt[:], axis=mybir.AxisListType.X,
                                            op=mybir.AluOpType.add)
                    negmu = pc.tile([P, 1], F32, tag="negmu")
                    nc.vector.tensor_scalar(out=negmu[:], in0=s1[:], scalar1=-1.0 / F,
                                            scalar2=None, op0=mybir.AluOpType.mult)
                    xc = pc.tile([P, F], F32, tag="xc")
                    nc.scalar.activation(out=xc[:], in_=xt[:],
                                         func=mybir.ActivationFunctionType.Identity,
                                         bias=negmu[:, 0:1])
                    scr = pc.tile([P, F], F32, tag="scr")
                    ss = pc.tile([P, 1], F32, tag="ss")
                    nc.scalar.activation(out=scr[:], in_=xc[:],
                                         func=mybir.ActivationFunctionType.Square,
                                         accum_out=ss[:])
                    v = pc.tile([P, 1], F32, tag="v")
                    nc.vector.tensor_scalar(out=v[:], in0=ss[:], scalar1=1.0 / F,
                                            scalar2=LN_EPS, op0=mybir.AluOpType.mult,
                                            op1=mybir.AluOpType.add)
                    sv = pc.tile([P, 1], F32, tag="sv")
                    nc.scalar.activation(out=sv[:], in_=v[:],
                                         func=mybir.ActivationFunctionType.Sqrt)
                    rstd = pc.tile([P, 1], F32, tag="rstd")
                    nc.vector.reciprocal(out=rstd[:], in_=sv[:])
                    t1 = pc.tile([P, F], F32, tag="t1")
                    nc.vector.scalar_tensor_tensor(out=t1[:], in0=xc[:], scalar=rstd[:, 0:1],
                                                   in1=vecs_t[:, 1, :],
                                                   op0=mybir.AluOpType.mult,
                                                   op1=mybir.AluOpType.mult)
                    y = pc.tile([P, F], F32, tag="y")
                    nc.vector.tensor_tensor(out=y[:], in0=t1[:], in1=vecs_t[:, 2, :],
                                            op=mybir.AluOpType.add)
                    nc.sync.dma_start(out=out_p[w * P:(w + 1) * P, :], in_=y[:])

    nc.compile()
    return nc


# ---------------- host side ----------------
def host_prep(h, src, dst, fc_w, attn_l, attn_r, gat_bias, out_w, out_b, ln_g, ln_b):
    h = np.ascontiguousarray(np.asarray(h, np.float32))
    src = np.asarray(src, np.int64)
    dst = np.asarray(dst, np.int64)
    fc_w = np.asarray(fc_w, np.float32)
    attn_l = np.asarray(attn_l, np.float32)
    attn_r = np.asarray(attn_r, np.float32)
    gat_bias = np.asarray(gat_bias, np.float32)
    out_w = np.asarray(out_w, np.float32)
    out_b = np.asarray(out_b, np.float32)
    ln_g = np.asarray(ln_g, np.float32)
    ln_b = np.asarray(ln_b, np.float32)

    A_l = np.einsum('khf,hf->kh', fc_w.reshape(F, H, F), attn_l).astype(np.float32)
    A_r = np.einsum('khf,hf->kh', fc_w.reshape(F, H, F), attn_r).astype(np.float32)
    Waug = np.ascontiguousarray(np.concatenate([fc_w, A_l, A_r], axis=1))  # [64, 264]
    bias2 = (gat_bias @ out_w + out_b).astype(np.float32)                  # [64]

    hT = np.zeros((F, GROWS), np.float32)
    hT[:, :N] = h.T
    hTo = np.zeros((NCORES, F, ERROWS), np.float32)
    for c in range(NCORES):
        hTo[c, :, :NPC] = h[c * NPC:(c + 1) * NPC].T

    vecs = np.zeros((P, 3, F), np.float32)
    vecs[:, 0, :] = bias2
    vecs[:, 1, :] = ln_g
    vecs[:, 2, :] = ln_b

    # sort edges by dst
    order = np.argsort(dst, kind='stable')
    ssrc = src[order]
    sdst = dst[order]
    core_of = sdst // NPC
    loc = sdst - core_of * NPC
    win = loc // P
    dloc = (loc - win * P).astype(np.float32)
    gw = core_of * NWIN + win
    counts = np.bincount(gw, minlength=NCORES * NWIN)
    starts = np.zeros(NCORES * NWIN + 1, np.int64)
    np.cumsum(counts, out=starts[1:])

    # per-(core,window) lo/hi sizes
    lomask = ssrc < LO
    CLs = np.zeros(NCORES * NWIN, np.int64)
    CHs = np.zeros(NCORES * NWIN, np.int64)
    for g in range(NCORES * NWIN):
        sl = slice(starts[g], starts[g + 1])
        nlo = int(lomask[sl].sum())
        nhi = int(counts[g] - nlo)
        CLs[g] = (nlo + P - 1) // P
        CHs[g] = (nhi + P - 1) // P
    CL = max(1, int(CLs.max()))
    CH = max(1, int(CHs.max()))
    CPW = CL + CH

    srcq = np.zeros((NCORES, P, NWIN * CPW * 8), np.int16)
    dstl = np.zeros((NCORES, P, NWIN * CPW * 8), np.int16)
    dstf = np.full((NCORES, P, NWIN * CPW), 200.0, np.float32)

    for g in range(NCORES * NWIN):
        c, w = divmod(g, NWIN)
        sl = slice(starts[g], starts[g + 1])
        s_src = ssrc[sl]; s_dl = dloc[sl]
        m = lomask[sl]
        lo_src, lo_dl = s_src[m], s_dl[m]
        hi_src, hi_dl = s_src[~m], s_dl[~m]
        nlo, nhi = len(lo_src), len(hi_src)
        sq = np.zeros(CPW * P, np.int16)
        dl = np.zeros(CPW * P, np.int16)
        df = np.full(CPW * P, 200.0, np.float32)
        sq[:nlo] = lo_src
        sq[CL * P:CL * P + nhi] = hi_src - LO
        dl[:nlo] = (w * P + lo_dl).astype(np.int16)
        dl[CL * P:CL * P + nhi] = (w * P + hi_dl).astype(np.int16)
        df[:nlo] = lo_dl
        df[CL * P:CL * P + nhi] = hi_dl
        wrap_s = np.tile(sq.reshape(CPW * 8, 16).T, (8, 1))
        wrap_d = np.tile(dl.reshape(CPW * 8, 16).T, (8, 1))
        srcq[c][:, w * CPW * 8:(w + 1) * CPW * 8] = wrap_s
        dstl[c][:, w * CPW * 8:(w + 1) * CPW * 8] = wrap_d
        dstf[c][:, w * CPW:(w + 1) * CPW] = df.reshape(CPW, P).T

    small = dict(Waug=Waug, outw=np.ascontiguousarray(out_w), vecs=vecs)
    return hT, hTo, srcq, dstl, dstf, small, CL, CH


_prog_cache = {}

def kernel(**inputs):
    hT, hTo, srcq, dstl, dstf, small, CL, CH = host_prep(**inputs)
    key = (CL, CH)
    if key not in _prog_cache:
        _prog_cache[key] = build_program(CL, CH)
    nc = _prog_cache[key]
    in_maps = []
    for c in range(NCORES):
        in_maps.append({
            "hT": hT, "hTo": hTo[c], "Waug": small["Waug"], "outw": small["outw"],
            "vecs": small["vecs"], "srcq": srcq[c], "dstl": dstl[c], "dstf": dstf[c],
        })
    res = run_bass_kernel_spmd(nc, in_maps, list(range(NCORES)))
    out = np.concatenate([np.asarray(res.results[c]["out"])[:NPC] for c in range(NCORES)], axis=0)
    return out
